# revision 64
# baseline (speedup 1.0000x reference)
"""Trainium2 Bass kernel for nn_MultiHeadAttentionLayer (edge-wise MHA with
global softmax over the edge dimension).

Strategy (8 NeuronCores, data-parallel over edges):
  - Host shards E=250000 edges into 8 shards of 31250, zero-padded to 31744
    (62 chunks x 512 = 31 pairs x 1024), pre-transposed so features land on
    SBUF partitions, ALL inputs cast to fp8e4 (output accuracy is dominated
    by the bias-scale terms, so the whole data path tolerates fp8).
  - All pass-A matmuls are fp8 DoubleRow, weights pre-scaled x4 per side so
    fp8 weight values stay out of the subnormal zone:
      Q'  = [4wq;0].T @ [xi8 | junk]            (zero weights kill junk)
      KE' = [4wk | 4we;4(bk+be);0].T @ [xj8 | ea8;1;0]   two mms in one
      V'  = [4wv | 4bv-row].T @ [xj8 | ea8;1;0] (same rhs as KE')
      S'  = [HsumRep;0 / 0;HsumRep].T @ P'      per-head sums replicated to
                                                all 16 lanes of each head
    P' = drain(Q'+4bq) * KE' (DVE pair-wide tensor_tensor, fp8 out); exp
    scale 1/64 undoes the 4x-per-side weight scaling and applies 1/sqrt(dk).
  - Engine split per pair: ACT does the Q' drain (+bias) and EXP (+Z accum);
    DVE does P' and U = exp * V' fused with V's PSUM drain.  The ea ones-row
    carries bk+be into KE' and 4bv into V', so no separate bias ops exist.
  - PSUM: four pair-wide [128,1024] f32 tiles (q, ke, v, s) = 8 banks, all
    single-buffered; the previous pair's S/EXP/U are emitted before the next
    pair's matmuls so the tile ring can order writers against readers.
  - Z: per-pair partials via EXP accum; a partial AllReduce over the first
    ARSPLIT pairs fires mid pass A (absorbs CC-stream startup + launch
    skew), a tail AllGather + local sum covers the rest; 1/Z and the /4
    from wv's prescale fold into wo2 = (wo/4)/Z rows.  Dummy matmuls keep
    the PE p-state hot across the collective gap.
  - Pass B: outT = wo2.T @ U + bo -> DRAM fp16, pair-granule PSUM tiles
    (bufs=4), drain copies alternate ACT/DVE per pair.
"""
import os
import sys

for _p in ("/opt/trn_rl_repo", "/root/.axon_site/_ro/trn_rl_repo"):
    if os.path.isdir(_p) and _p not in sys.path:
        sys.path.append(_p)

import numpy as np
import ml_dtypes
import concourse.bacc as bacc
import concourse.tile as tile
import concourse.mybir as mybir
from concourse.bass_utils import run_bass_kernel_spmd

F32 = mybir.dt.float32
BF16 = mybir.dt.bfloat16
FP8 = mybir.dt.float8e4
FP16 = mybir.dt.float16
AF = mybir.ActivationFunctionType
ALU = mybir.AluOpType
PM = mybir.MatmulPerfMode.DoubleRow
BF = ml_dtypes.bfloat16
F8 = ml_dtypes.float8_e4m3

E_FULL = 250000
NCORES = 8
ES = E_FULL // NCORES          # 31250 edges per core
CH = 512                       # chunk size (PSUM bank width)
NCH = (ES + CH - 1) // CH      # 62 chunks
EP = NCH * CH                  # 31744 padded edges per core
D = 128
NH = 8
DK = 16
PR = 2 * CH                    # pair width (1024)
NPAIR = NCH // 2               # 31 pairs
XW = 2048                      # xi/xj DMA batch width (4 chunks)
KSUB = XW // CH                # chunks per batch (4)
PTAIL = ES - (NPAIR - 1) * PR  # valid edges in last pair (530)
ARSPLIT = 8                    # Z pairs in the early (hidden) AllReduce
WS = 4.0                       # fp8 weight pre-scale per side
EXPSCALE = 0.25 / (WS * WS)    # undo both sides + 1/sqrt(dk)

_CACHE = {}


def _build():
    if "nc" in _CACHE:
        return _CACHE["nc"]

    nc = bacc.Bacc(num_devices=NCORES)

    t_xi8 = nc.dram_tensor("xi8", [D, EP], FP8, kind="ExternalInput")
    t_xj8 = nc.dram_tensor("xj8", [D, EP], FP8, kind="ExternalInput")
    t_ea33 = nc.dram_tensor("ea33", [33, EP], FP8, kind="ExternalInput")
    t_pk8 = nc.dram_tensor("pk8", [D, 1536], FP8, kind="ExternalInput")
    t_pkb = nc.dram_tensor("pkb", [D, 256], BF16, kind="ExternalInput")
    t_pkf = nc.dram_tensor("pkf", [D, 8], F32, kind="ExternalInput")
    t_out = nc.dram_tensor("outT", [D, EP], FP16, kind="ExternalOutput")

    with tile.TileContext(nc) as tc:
        with (
            tc.tile_pool(name="per", bufs=1) as per,      # persistent
            tc.tile_pool(name="wk", bufs=3) as wk,        # streaming loads
            tc.tile_pool(name="mid", bufs=2) as mid,      # intermediates
            tc.tile_pool(name="dram", bufs=1, space="DRAM") as dram,
        ):
            s_pk8 = per.tile([D, 1536], FP8)
            nc.sync.dma_start(s_pk8[:], t_pk8[:])
            w_q0 = s_pk8[:, 0:256].rearrange("p (s m) -> p s m", s=2)
            w_q1 = s_pk8[:, 256:512].rearrange("p (s m) -> p s m", s=2)
            w_ke = s_pk8[:, 512:768].rearrange("p (s m) -> p s m", s=2)
            w_hs0 = s_pk8[:, 768:1024].rearrange("p (s m) -> p s m", s=2)
            w_hs1 = s_pk8[:, 1024:1280].rearrange("p (s m) -> p s m", s=2)
            w_v0 = s_pk8[:, 1280:1536].rearrange("p (s m) -> p s m", s=2)

            s_pkb = per.tile([D, 256], BF16)
            nc.sync.dma_start(s_pkb[:], t_pkb[:])
            s_wo = s_pkb[:, 128:256]   # wo/4 (undoes the 4x on wv)

            s_pkf = per.tile([D, 8], F32)
            nc.sync.dma_start(s_pkf[:], t_pkf[:])
            s_bq = s_pkf[:, 0:1]       # 4*bq
            s_bo = s_pkf[:, 1:2]       # bo

            v_full = per.tile([D, EP], BF16)     # U = exp * V resident
            e_full = per.tile([D, EP], BF16)     # resident exp, replicated
            zparts = per.tile([D, NPAIR], F32)   # per-pair Z partials

            s_z1 = per.tile([D, 1], F32)
            d_zin1 = dram.tile([D, 1], F32)
            d_zout1 = dram.tile([NCORES, D], F32)

            # ---------------- pass A ----------------
            psA_ctx = tc.tile_pool(name="psA", bufs=1, space="PSUM")
            psA = psA_ctx.__enter__()
            warm = per.tile([D, CH], BF16)
            nc.vector.memset(warm[:], 0.0)
            # pre-zero the ea side-channel region of the 3 xje ring buffers
            # (partitions 33:128 of the second subtile feed the DR matmul
            # against zero weights and must be finite)
            for _b in range(3):
                t0 = wk.tile([D, 2 * XW], FP8, tag="xje", name=f"xjez{_b}")
                eng = (nc.vector, nc.gpsimd, nc.gpsimd)[_b]
                eng.memset(t0[:, XW:2 * XW], 0.0)
            p_warm = psA.tile([D, PR], F32, tag="pq", bufs=1, name="p_warm")
            for _ in range(16):
                nc.tensor.matmul(p_warm[:, 0:CH], warm[:, 0:128], warm[:],
                                 start=True, stop=True)

            def do_tail(p, p_s, p_v, s_p):
                # S-DR pair, EXP, U for pair p (p_s/p_v/s_p are its tiles)
                sl2 = slice(p * PR, (p + 1) * PR)
                sp3 = s_p[:].rearrange("p (s n) -> p s n", s=2)
                nc.tensor.matmul(p_s[:, 0:CH], w_hs0, sp3,
                                 start=True, stop=True, perf_mode=PM)
                nc.tensor.matmul(p_s[:, CH:PR], w_hs1, sp3,
                                 start=True, stop=True, perf_mode=PM)
                if p < NPAIR - 1:
                    nc.scalar.activation(e_full[:, sl2], p_s[:], AF.Exp,
                                         bias=0.0, scale=EXPSCALE,
                                         accum_out=zparts[:, p:p + 1])
                else:
                    nc.scalar.activation(e_full[:, sl2], p_s[:], AF.Exp,
                                         bias=0.0, scale=EXPSCALE)
                    nc.vector.memset(
                        e_full[:, p * PR + PTAIL:(p + 1) * PR], 0.0)
                    nc.vector.tensor_reduce(zparts[:, p:p + 1],
                                            e_full[:, sl2],
                                            axis=mybir.AxisListType.X,
                                            op=ALU.add)
                # U = exp * (V + bv) straight out of PSUM (fused V drain;
                # 4bv already added by the ea ones-row in the V matmul)
                nc.vector.tensor_tensor(v_full[:, sl2], e_full[:, sl2],
                                        p_v[:], op=ALU.mult)

            pend = None   # (p, p_s, p_v, s_p) waiting for tail ops
            for p in range(NPAIR):
                c0 = 2 * p            # even chunk index
                if c0 % KSUB == 0:
                    xw = min(XW, EP - c0 * CH)
                    s_xi = wk.tile([D, XW], FP8, tag="xi")
                    nc.sync.dma_start(s_xi[:, :xw], t_xi8[:, c0 * CH:c0 * CH + xw])
                    s_xje = wk.tile([D, 2 * XW], FP8, tag="xje")
                    nc.sync.dma_start(s_xje[:, :xw], t_xj8[:, c0 * CH:c0 * CH + xw])
                    nc.sync.dma_start(s_xje[0:33, XW:XW + xw],
                                      t_ea33[:, c0 * CH:c0 * CH + xw])

                kb = c0 % KSUB        # chunk-in-batch (0 or 2)

                # tail of the previous pair first: its S/EXP/U must be on
                # the queues before this pair's allocations reuse the
                # single-buffered PSUM tiles (the tile ring can only order
                # writers against readers already emitted)
                if pend is not None:
                    do_tail(*pend)
                    pend = None

                p_q = psA.tile([D, PR], F32, tag="pq", bufs=1)
                p_ke = psA.tile([D, PR], F32, tag="pke", bufs=1)
                p_v = psA.tile([D, PR], F32, tag="pv", bufs=1)
                # Q': DR against the junk half of the xi batch (zero weights
                # select the real subtile); KE'/V' share one DR rhs
                xi3 = s_xi[:, 0:2 * PR].rearrange("p (s n) -> p s n", s=2)
                wsel = w_q0 if kb == 0 else w_q1
                xje3 = s_xje[:, :].rearrange("p (s w) -> p s w", s=2)
                for h in range(2):
                    k = kb + h
                    nc.tensor.matmul(p_q[:, h * CH:(h + 1) * CH], wsel,
                                     xi3[:, :, h * CH:(h + 1) * CH],
                                     start=True, stop=True, perf_mode=PM)
                    nc.tensor.matmul(p_ke[:, h * CH:(h + 1) * CH], w_ke,
                                     xje3[:, :, k * CH:(k + 1) * CH],
                                     start=True, stop=True, perf_mode=PM)

                # drain Q' (+4bq) to SBUF bf16 on ACT
                s_q = mid.tile([D, PR], BF16, tag="q", bufs=3)
                nc.scalar.activation(s_q[:], p_q[:], AF.Identity,
                                     bias=s_bq, scale=1.0)
                # P' = drained(Q') * KE'  (pair-wide, fp8 out)
                s_p = mid.tile([D, PR], FP8, tag="p", bufs=3)
                nc.vector.tensor_tensor(s_p[:], p_ke[:], s_q[:], op=ALU.mult)

                # V' last: maximizes slack for U(p-1)'s read of the
                # single-buffered p_v ring
                for h in range(2):
                    k = kb + h
                    nc.tensor.matmul(p_v[:, h * CH:(h + 1) * CH], w_v0,
                                     xje3[:, :, k * CH:(k + 1) * CH],
                                     start=True, stop=True, perf_mode=PM)

                p_s = psA.tile([D, PR], F32, tag="ps", bufs=1)
                pend = (p, p_s, p_v, s_p)

                if p == ARSPLIT:
                    nc.vector.tensor_reduce(s_z1[:], zparts[:, :ARSPLIT],
                                            axis=mybir.AxisListType.X,
                                            op=ALU.add)
                    nc.sync.dma_start(d_zin1[:], s_z1[:])
                    nc.gpsimd.collective_compute(
                        "AllGather", ALU.bypass,
                        replica_groups=[list(range(NCORES))],
                        ins=[d_zin1.opt()],
                        outs=[d_zout1.opt()],
                    )
            do_tail(*pend)

            psA_ctx.__exit__(None, None, None)
            psB_ctx = tc.tile_pool(name="psB", bufs=1, space="PSUM")
            psB = psB_ctx.__enter__()

            # ---------------- global Z ----
            s_z2 = per.tile([D, 1], F32)
            nc.vector.tensor_reduce(s_z2[:], zparts[:, ARSPLIT:],
                                    axis=mybir.AxisListType.X, op=ALU.add)
            d_zin2 = dram.tile([D, 1], F32)
            d_zout2 = dram.tile([NCORES, D], F32)
            nc.sync.dma_start(d_zin2[:], s_z2[:])
            nc.gpsimd.collective_compute(
                "AllGather", ALU.bypass,
                replica_groups=[list(range(NCORES))],
                ins=[d_zin2.opt()],
                outs=[d_zout2.opt()],
            )
            # keep the PE p-state hot while the tail collective drains
            p_keep = psB.tile([D, PR], F32, tag="pout", bufs=4,
                              name="p_keep")
            for _ in range(110):
                nc.tensor.matmul(p_keep[:, 0:128], warm[:, 0:128],
                                 warm[:, 0:128], start=True, stop=True)
            s_zg1 = per.tile([D, NCORES], F32)
            nc.sync.dma_start(s_zg1[:], d_zout1[:].rearrange("r d -> d r"))
            s_za = per.tile([D, 1], F32)
            nc.vector.tensor_reduce(s_za[:], s_zg1[:],
                                    axis=mybir.AxisListType.X, op=ALU.add)
            s_zg = per.tile([D, NCORES], F32)
            nc.sync.dma_start(s_zg[:], d_zout2[:].rearrange("r d -> d r"))
            s_z2s = per.tile([D, 1], F32)
            nc.vector.tensor_reduce(s_z2s[:], s_zg[:],
                                    axis=mybir.AxisListType.X, op=ALU.add)
            s_zsum = per.tile([D, 1], F32)
            nc.vector.tensor_tensor(s_zsum[:], s_za[:], s_z2s[:], op=ALU.add)
            s_chd = per.tile([D, 1], F32)
            nc.vector.reciprocal(s_chd[:], s_zsum[:])
            s_wo2 = per.tile([D, D], BF16)
            nc.vector.tensor_scalar(s_wo2[:], s_wo, s_chd[:], None,
                                    op0=ALU.mult)

            # ---------------- pass B (pair-granule stores) ----------------
            for q in range(NPAIR):
                sl2 = slice(q * PR, (q + 1) * PR)
                p_o = psB.tile([D, PR], F32, tag="pout", bufs=4,
                               name=f"po_{q}")
                for h in range(2):
                    hs = slice((2 * q + h) * CH, (2 * q + h + 1) * CH)
                    nc.tensor.matmul(p_o[:, h * CH:(h + 1) * CH], s_wo2[:],
                                     v_full[:, hs], start=True, stop=True)
                s_o = mid.tile([D, PR], FP16, tag="o", bufs=6)
                if q % 2 == 0:
                    nc.scalar.activation(s_o[:], p_o[:], AF.Identity,
                                         bias=s_bo, scale=1.0)
                else:
                    nc.vector.tensor_scalar(s_o[:], p_o[:], s_bo, None,
                                            op0=ALU.add)
                nc.sync.dma_start(t_out[:, sl2], s_o[:])
            psB_ctx.__exit__(None, None, None)

    nc.compile()
    _CACHE["nc"] = nc
    return nc


def _pack_constants(wq, bq, wk, bk, wv, bv, we, be, wo, bo):
    HsumRep = np.zeros((D, D), np.float32)   # [f, hd] = (head(f)==head(hd))
    for f in range(D):
        h = f // DK
        HsumRep[f, h * DK:(h + 1) * DK] = 1.0
    pk8 = np.zeros((D, 1536), np.float32)
    pk8[:, 0:128] = WS * wq          # w_q0 = [4wq ; 0]
    pk8[:, 384:512] = WS * wq        # w_q1 = [0 ; 4wq]
    pk8[:, 512:640] = WS * wk        # w_ke subtile 0
    pk8[0:32, 640:768] = WS * we     # w_ke subtile 1 rows 0:32
    pk8[32, 640:768] = WS * (bk + be)  # ones-row bias
    pk8[:, 768:896] = HsumRep        # w_hs0 = [H ; 0]
    pk8[:, 1152:1280] = HsumRep      # w_hs1 = [0 ; H]
    pk8[:, 1280:1408] = WS * wv      # w_v0 = [4wv ; bv-row]
    pk8[32, 1408:1536] = WS * bv     # ea ones-row adds 4bv to V'
    pkb = np.zeros((D, 256), np.float32)
    pkb[:, 128:256] = wo / WS        # undoes the 4x on wv
    pkf = np.zeros((D, 8), np.float32)
    pkf[:, 0] = WS * bq
    pkf[:, 1] = bo
    return pk8.astype(F8), pkb.astype(BF), pkf


def _run(inputs, trace=False):
    x_i = np.asarray(inputs["x_i"], np.float32)
    x_j = np.asarray(inputs["x_j"], np.float32)
    ea = np.asarray(inputs["edge_attr"], np.float32)
    pk8, pkb, pkf = _pack_constants(
        np.asarray(inputs["wq"], np.float32), np.asarray(inputs["bq"], np.float32),
        np.asarray(inputs["wk"], np.float32), np.asarray(inputs["bk"], np.float32),
        np.asarray(inputs["wv"], np.float32), np.asarray(inputs["bv"], np.float32),
        np.asarray(inputs["we"], np.float32), np.asarray(inputs["be"], np.float32),
        np.asarray(inputs["wo"], np.float32), np.asarray(inputs["bo"], np.float32),
    )

    in_maps = []
    for c in range(NCORES):
        sl = slice(c * ES, (c + 1) * ES)
        xi8 = np.zeros((D, EP), F8)
        xi8[:, :ES] = x_i[sl].T.astype(F8)
        xj8 = np.zeros((D, EP), F8)
        xj8[:, :ES] = x_j[sl].T.astype(F8)
        ea33 = np.zeros((33, EP), F8)
        ea33[:32, :ES] = ea[sl].T.astype(F8)
        ea33[32, :ES] = 1.0
        in_maps.append(dict(xi8=xi8, xj8=xj8, ea33=ea33,
                            pk8=pk8, pkb=pkb, pkf=pkf))

    nc = _build()
    res = run_bass_kernel_spmd(nc, in_maps, list(range(NCORES)), trace=trace)

    out = np.empty((E_FULL, D), np.float32)
    for c in range(NCORES):
        sl = slice(c * ES, (c + 1) * ES)
        out[sl] = res.results[c]["outT"][:, :ES].T.astype(np.float32)
    return out, res.exec_time_ns


def kernel(**inputs) -> np.ndarray:
    return _run(inputs)[0]


# revision 65
# speedup vs baseline: 1.1498x; 1.1498x over previous
"""Trainium2 Bass kernel for nn_MultiHeadAttentionLayer (edge-wise MHA with
global softmax over the edge dimension).

Strategy (8 NeuronCores, data-parallel over edges):
  - Host shards E=250000 edges into 8 shards of 31250, zero-padded to 31744
    (62 chunks x 512 = 31 pairs x 1024), pre-transposed so features land on
    SBUF partitions, ALL inputs cast to fp8e4 (output accuracy is dominated
    by the bias-scale terms, so the whole data path tolerates fp8).
  - All pass-A matmuls are fp8 DoubleRow, weights pre-scaled x4 per side so
    fp8 weight values stay out of the subnormal zone:
      Q'  = [4wq;0].T @ [xi8 | junk]            (zero weights kill junk)
      KE' = [4wk | 4we;4(bk+be);0].T @ [xj8 | ea8;1;0]   two mms in one
      V'  = [4wv | 4bv-row].T @ [xj8 | ea8;1;0] (same rhs as KE')
      S'  = [HsumRep;0 / 0;HsumRep].T @ P'      per-head sums replicated to
                                                all 16 lanes of each head
    P' = drain(Q'+4bq) * KE' (DVE pair-wide tensor_tensor, fp8 out); exp
    scale 1/64 undoes the 4x-per-side weight scaling and applies 1/sqrt(dk).
  - Engine split per pair: ACT does the Q' drain (+bias) and EXP (+Z accum);
    DVE does P' and U = exp * V' fused with V's PSUM drain.  The ea ones-row
    carries bk+be into KE' and 4bv into V', so no separate bias ops exist.
  - PSUM: four pair-wide [128,1024] f32 tiles (q, ke, v, s) = 8 banks, all
    single-buffered; the previous pair's S/EXP/U are emitted before the next
    pair's matmuls so the tile ring can order writers against readers.
  - Z: per-pair partials via EXP accum; a partial AllReduce over the first
    ARSPLIT pairs fires mid pass A (absorbs CC-stream startup + launch
    skew), a tail AllGather + local sum covers the rest; 1/Z and the /4
    from wv's prescale fold into wo2 = (wo/4)/Z rows.  Dummy matmuls keep
    the PE p-state hot across the collective gap.
  - Pass B: outT = wo2.T @ U + bo -> DRAM fp16, pair-granule PSUM tiles
    (bufs=4), drain copies alternate ACT/DVE per pair.
"""
import os
import sys

for _p in ("/opt/trn_rl_repo", "/root/.axon_site/_ro/trn_rl_repo"):
    if os.path.isdir(_p) and _p not in sys.path:
        sys.path.append(_p)

import numpy as np
import ml_dtypes
import concourse.bacc as bacc
import concourse.tile as tile
import concourse.mybir as mybir
from concourse.bass_utils import run_bass_kernel_spmd

F32 = mybir.dt.float32
BF16 = mybir.dt.bfloat16
FP8 = mybir.dt.float8e4
FP16 = mybir.dt.float16
AF = mybir.ActivationFunctionType
ALU = mybir.AluOpType
PM = mybir.MatmulPerfMode.DoubleRow
BF = ml_dtypes.bfloat16
F8 = ml_dtypes.float8_e4m3

E_FULL = 250000
NCORES = 8
ES = E_FULL // NCORES          # 31250 edges per core
CH = 512                       # chunk size (PSUM bank width)
NCH = (ES + CH - 1) // CH      # 62 chunks
EP = NCH * CH                  # 31744 padded edges per core
D = 128
NH = 8
DK = 16
PR = 2 * CH                    # pair width (1024)
NPAIR = NCH // 2               # 31 pairs
XW = 2048                      # xi/xj DMA batch width (4 chunks)
KSUB = XW // CH                # chunks per batch (4)
PTAIL = ES - (NPAIR - 1) * PR  # valid edges in last pair (530)
ARSPLIT = 8                    # Z pairs in the early (hidden) AllReduce
WS = 4.0                       # fp8 weight pre-scale per side
EXPSCALE = 0.25 / (WS * WS)    # undo both sides + 1/sqrt(dk)

_CACHE = {}


def _build():
    if "nc" in _CACHE:
        return _CACHE["nc"]

    nc = bacc.Bacc(num_devices=NCORES)

    t_xi8 = nc.dram_tensor("xi8", [D, EP], FP8, kind="ExternalInput")
    t_xj8 = nc.dram_tensor("xj8", [D, EP], FP8, kind="ExternalInput")
    t_ea33 = nc.dram_tensor("ea33", [33, EP], FP8, kind="ExternalInput")
    t_pk8 = nc.dram_tensor("pk8", [D, 1536], FP8, kind="ExternalInput")
    t_pkb = nc.dram_tensor("pkb", [D, 256], BF16, kind="ExternalInput")
    t_pkf = nc.dram_tensor("pkf", [D, 8], F32, kind="ExternalInput")
    t_out = nc.dram_tensor("outT", [D, EP], FP16, kind="ExternalOutput")

    with tile.TileContext(nc) as tc:
        with (
            tc.tile_pool(name="per", bufs=1) as per,      # persistent
            tc.tile_pool(name="wk", bufs=3) as wk,        # streaming loads
            tc.tile_pool(name="mid", bufs=2) as mid,      # intermediates
            tc.tile_pool(name="dram", bufs=1, space="DRAM") as dram,
        ):
            s_pk8 = per.tile([D, 1536], FP8)
            nc.sync.dma_start(s_pk8[:], t_pk8[:])
            w_q0 = s_pk8[:, 0:256].rearrange("p (s m) -> p s m", s=2)
            w_q1 = s_pk8[:, 256:512].rearrange("p (s m) -> p s m", s=2)
            w_ke = s_pk8[:, 512:768].rearrange("p (s m) -> p s m", s=2)
            w_hs0 = s_pk8[:, 768:1024].rearrange("p (s m) -> p s m", s=2)
            w_hs1 = s_pk8[:, 1024:1280].rearrange("p (s m) -> p s m", s=2)
            w_v0 = s_pk8[:, 1280:1536].rearrange("p (s m) -> p s m", s=2)

            s_pkb = per.tile([D, 256], BF16)
            nc.sync.dma_start(s_pkb[:], t_pkb[:])
            s_wo = s_pkb[:, 128:256]   # wo/4 (undoes the 4x on wv)

            s_pkf = per.tile([D, 8], F32)
            nc.sync.dma_start(s_pkf[:], t_pkf[:])
            s_bq = s_pkf[:, 0:1]       # 4*bq
            s_bo = s_pkf[:, 1:2]       # bo

            v_full = per.tile([D, EP], BF16)     # U = exp * V resident
            e_full = per.tile([D, EP], BF16)     # resident exp, replicated
            zparts = per.tile([D, NPAIR], F32)   # per-pair Z partials

            s_z1 = per.tile([D, 1], F32)
            d_zin1 = dram.tile([D, 1], F32)
            d_zout1 = dram.tile([D, 1], F32)

            # ---------------- pass A ----------------
            psA_ctx = tc.tile_pool(name="psA", bufs=1, space="PSUM")
            psA = psA_ctx.__enter__()
            warm = per.tile([D, CH], BF16)
            nc.vector.memset(warm[:], 0.0)
            # pre-zero the ea side-channel region of the 3 xje ring buffers
            # (partitions 33:128 of the second subtile feed the DR matmul
            # against zero weights and must be finite)
            for _b in range(3):
                t0 = wk.tile([D, 2 * XW], FP8, tag="xje", name=f"xjez{_b}")
                eng = (nc.vector, nc.gpsimd, nc.gpsimd)[_b]
                eng.memset(t0[:, XW:2 * XW], 0.0)
            p_warm = psA.tile([D, PR], F32, tag="pq", bufs=1, name="p_warm")
            for _ in range(28):
                nc.tensor.matmul(p_warm[:, 0:CH], warm[:, 0:128], warm[:],
                                 start=True, stop=True)

            def do_tail(p, p_s, p_v, s_p):
                # S-DR pair, EXP, U for pair p (p_s/p_v/s_p are its tiles)
                sl2 = slice(p * PR, (p + 1) * PR)
                sp3 = s_p[:].rearrange("p (s n) -> p s n", s=2)
                nc.tensor.matmul(p_s[:, 0:CH], w_hs0, sp3,
                                 start=True, stop=True, perf_mode=PM)
                nc.tensor.matmul(p_s[:, CH:PR], w_hs1, sp3,
                                 start=True, stop=True, perf_mode=PM)
                if p < NPAIR - 1:
                    nc.scalar.activation(e_full[:, sl2], p_s[:], AF.Exp,
                                         bias=0.0, scale=EXPSCALE,
                                         accum_out=zparts[:, p:p + 1])
                else:
                    nc.scalar.activation(e_full[:, sl2], p_s[:], AF.Exp,
                                         bias=0.0, scale=EXPSCALE)
                    nc.vector.memset(
                        e_full[:, p * PR + PTAIL:(p + 1) * PR], 0.0)
                    nc.vector.tensor_reduce(zparts[:, p:p + 1],
                                            e_full[:, sl2],
                                            axis=mybir.AxisListType.X,
                                            op=ALU.add)
                # U = exp * (V + bv) straight out of PSUM (fused V drain;
                # 4bv already added by the ea ones-row in the V matmul)
                nc.vector.tensor_tensor(v_full[:, sl2], e_full[:, sl2],
                                        p_v[:], op=ALU.mult)

            pend = None   # (p, p_s, p_v, s_p) waiting for tail ops
            for p in range(NPAIR):
                c0 = 2 * p            # even chunk index
                if c0 % KSUB == 0:
                    xw = min(XW, EP - c0 * CH)
                    s_xi = wk.tile([D, XW], FP8, tag="xi")
                    nc.sync.dma_start(s_xi[:, :xw], t_xi8[:, c0 * CH:c0 * CH + xw])
                    s_xje = wk.tile([D, 2 * XW], FP8, tag="xje")
                    nc.sync.dma_start(s_xje[:, :xw], t_xj8[:, c0 * CH:c0 * CH + xw])
                    nc.sync.dma_start(s_xje[0:33, XW:XW + xw],
                                      t_ea33[:, c0 * CH:c0 * CH + xw])

                kb = c0 % KSUB        # chunk-in-batch (0 or 2)

                # tail of the previous pair first: its S/EXP/U must be on
                # the queues before this pair's allocations reuse the
                # single-buffered PSUM tiles (the tile ring can only order
                # writers against readers already emitted)
                if pend is not None:
                    do_tail(*pend)
                    pend = None

                p_q = psA.tile([D, PR], F32, tag="pq", bufs=1)
                p_ke = psA.tile([D, PR], F32, tag="pke", bufs=1)
                p_v = psA.tile([D, PR], F32, tag="pv", bufs=1)
                # Q': DR against the junk half of the xi batch (zero weights
                # select the real subtile); KE'/V' share one DR rhs
                xi3 = s_xi[:, 0:2 * PR].rearrange("p (s n) -> p s n", s=2)
                wsel = w_q0 if kb == 0 else w_q1
                xje3 = s_xje[:, :].rearrange("p (s w) -> p s w", s=2)
                for h in range(2):
                    k = kb + h
                    nc.tensor.matmul(p_q[:, h * CH:(h + 1) * CH], wsel,
                                     xi3[:, :, h * CH:(h + 1) * CH],
                                     start=True, stop=True, perf_mode=PM)
                    nc.tensor.matmul(p_ke[:, h * CH:(h + 1) * CH], w_ke,
                                     xje3[:, :, k * CH:(k + 1) * CH],
                                     start=True, stop=True, perf_mode=PM)

                # drain Q' (+4bq) to SBUF bf16 on ACT
                s_q = mid.tile([D, PR], BF16, tag="q", bufs=3)
                nc.scalar.activation(s_q[:], p_q[:], AF.Identity,
                                     bias=s_bq, scale=1.0)
                # P' = drained(Q') * KE'  (pair-wide, fp8 out)
                s_p = mid.tile([D, PR], FP8, tag="p", bufs=3)
                nc.vector.tensor_tensor(s_p[:], p_ke[:], s_q[:], op=ALU.mult)

                # V' last: maximizes slack for U(p-1)'s read of the
                # single-buffered p_v ring
                for h in range(2):
                    k = kb + h
                    nc.tensor.matmul(p_v[:, h * CH:(h + 1) * CH], w_v0,
                                     xje3[:, :, k * CH:(k + 1) * CH],
                                     start=True, stop=True, perf_mode=PM)

                p_s = psA.tile([D, PR], F32, tag="ps", bufs=1)
                pend = (p, p_s, p_v, s_p)

                if p == ARSPLIT:
                    nc.vector.tensor_reduce(s_z1[:], zparts[:, :ARSPLIT],
                                            axis=mybir.AxisListType.X,
                                            op=ALU.add)
                    nc.sync.dma_start(d_zin1[:], s_z1[:])
                    nc.gpsimd.collective_compute(
                        "AllReduce", ALU.add,
                        replica_groups=[list(range(NCORES))],
                        ins=[d_zin1.opt()],
                        outs=[d_zout1.opt()],
                    )
            do_tail(*pend)

            psA_ctx.__exit__(None, None, None)
            psB_ctx = tc.tile_pool(name="psB", bufs=1, space="PSUM")
            psB = psB_ctx.__enter__()

            # ---------------- global Z ----
            s_z2 = per.tile([D, 1], F32)
            nc.vector.tensor_reduce(s_z2[:], zparts[:, ARSPLIT:],
                                    axis=mybir.AxisListType.X, op=ALU.add)
            d_zin2 = dram.tile([D, 1], F32)
            d_zout2 = dram.tile([NCORES, D], F32)
            nc.sync.dma_start(d_zin2[:], s_z2[:])
            nc.gpsimd.collective_compute(
                "AllGather", ALU.bypass,
                replica_groups=[list(range(NCORES))],
                ins=[d_zin2.opt()],
                outs=[d_zout2.opt()],
            )
            # keep the PE p-state hot while the tail collective drains
            p_keep = psB.tile([D, PR], F32, tag="pout", bufs=4,
                              name="p_keep")
            for _ in range(110):
                nc.tensor.matmul(p_keep[:, 0:128], warm[:, 0:128],
                                 warm[:, 0:128], start=True, stop=True)
            s_za = per.tile([D, 1], F32)
            nc.sync.dma_start(s_za[:], d_zout1[:])
            s_zg = per.tile([D, NCORES], F32)
            nc.sync.dma_start(s_zg[:], d_zout2[:].rearrange("r d -> d r"))
            s_z2s = per.tile([D, 1], F32)
            nc.vector.tensor_reduce(s_z2s[:], s_zg[:],
                                    axis=mybir.AxisListType.X, op=ALU.add)
            s_zsum = per.tile([D, 1], F32)
            nc.vector.tensor_tensor(s_zsum[:], s_za[:], s_z2s[:], op=ALU.add)
            s_chd = per.tile([D, 1], F32)
            nc.vector.reciprocal(s_chd[:], s_zsum[:])
            s_wo2 = per.tile([D, D], BF16)
            nc.vector.tensor_scalar(s_wo2[:], s_wo, s_chd[:], None,
                                    op0=ALU.mult)

            # ---------------- pass B (pair-granule stores) ----------------
            for q in range(NPAIR):
                sl2 = slice(q * PR, (q + 1) * PR)
                p_o = psB.tile([D, PR], F32, tag="pout", bufs=4,
                               name=f"po_{q}")
                for h in range(2):
                    hs = slice((2 * q + h) * CH, (2 * q + h + 1) * CH)
                    nc.tensor.matmul(p_o[:, h * CH:(h + 1) * CH], s_wo2[:],
                                     v_full[:, hs], start=True, stop=True)
                s_o = mid.tile([D, PR], FP16, tag="o", bufs=6)
                if q % 2 == 0:
                    nc.scalar.activation(s_o[:], p_o[:], AF.Identity,
                                         bias=s_bo, scale=1.0)
                else:
                    nc.vector.tensor_scalar(s_o[:], p_o[:], s_bo, None,
                                            op0=ALU.add)
                nc.sync.dma_start(t_out[:, sl2], s_o[:])
            psB_ctx.__exit__(None, None, None)

    nc.compile()
    _CACHE["nc"] = nc
    return nc


def _pack_constants(wq, bq, wk, bk, wv, bv, we, be, wo, bo):
    HsumRep = np.zeros((D, D), np.float32)   # [f, hd] = (head(f)==head(hd))
    for f in range(D):
        h = f // DK
        HsumRep[f, h * DK:(h + 1) * DK] = 1.0
    pk8 = np.zeros((D, 1536), np.float32)
    pk8[:, 0:128] = WS * wq          # w_q0 = [4wq ; 0]
    pk8[:, 384:512] = WS * wq        # w_q1 = [0 ; 4wq]
    pk8[:, 512:640] = WS * wk        # w_ke subtile 0
    pk8[0:32, 640:768] = WS * we     # w_ke subtile 1 rows 0:32
    pk8[32, 640:768] = WS * (bk + be)  # ones-row bias
    pk8[:, 768:896] = HsumRep        # w_hs0 = [H ; 0]
    pk8[:, 1152:1280] = HsumRep      # w_hs1 = [0 ; H]
    pk8[:, 1280:1408] = WS * wv      # w_v0 = [4wv ; bv-row]
    pk8[32, 1408:1536] = WS * bv     # ea ones-row adds 4bv to V'
    pkb = np.zeros((D, 256), np.float32)
    pkb[:, 128:256] = wo / WS        # undoes the 4x on wv
    pkf = np.zeros((D, 8), np.float32)
    pkf[:, 0] = WS * bq
    pkf[:, 1] = bo
    return pk8.astype(F8), pkb.astype(BF), pkf


def _run(inputs, trace=False):
    x_i = np.asarray(inputs["x_i"], np.float32)
    x_j = np.asarray(inputs["x_j"], np.float32)
    ea = np.asarray(inputs["edge_attr"], np.float32)
    pk8, pkb, pkf = _pack_constants(
        np.asarray(inputs["wq"], np.float32), np.asarray(inputs["bq"], np.float32),
        np.asarray(inputs["wk"], np.float32), np.asarray(inputs["bk"], np.float32),
        np.asarray(inputs["wv"], np.float32), np.asarray(inputs["bv"], np.float32),
        np.asarray(inputs["we"], np.float32), np.asarray(inputs["be"], np.float32),
        np.asarray(inputs["wo"], np.float32), np.asarray(inputs["bo"], np.float32),
    )

    in_maps = []
    for c in range(NCORES):
        sl = slice(c * ES, (c + 1) * ES)
        xi8 = np.zeros((D, EP), F8)
        xi8[:, :ES] = x_i[sl].T.astype(F8)
        xj8 = np.zeros((D, EP), F8)
        xj8[:, :ES] = x_j[sl].T.astype(F8)
        ea33 = np.zeros((33, EP), F8)
        ea33[:32, :ES] = ea[sl].T.astype(F8)
        ea33[32, :ES] = 1.0
        in_maps.append(dict(xi8=xi8, xj8=xj8, ea33=ea33,
                            pk8=pk8, pkb=pkb, pkf=pkf))

    nc = _build()
    res = run_bass_kernel_spmd(nc, in_maps, list(range(NCORES)), trace=trace)

    out = np.empty((E_FULL, D), np.float32)
    for c in range(NCORES):
        sl = slice(c * ES, (c + 1) * ES)
        out[sl] = res.results[c]["outT"][:, :ES].T.astype(np.float32)
    return out, res.exec_time_ns


def kernel(**inputs) -> np.ndarray:
    return _run(inputs)[0]
